# revision 15
# baseline (speedup 1.0000x reference)
"""Multi-head attention (B=4, S=2048, D=1024, H=16) on 8 trn2 NeuronCores.

Sharding: batch (4-way) x head-half (2-way).  Core c = 2*b + hh handles batch b
and heads hh*8 .. hh*8+7.  v3: all matmul operands bf16 (halves DMA, enables
fast weight load), scheduled so neither the PE HAM clock-gate nor the
exp->attnv->scores semaphore loop limits the attention cadence:

  1. KT proj (full), QT proj (q-half 0), V proj (full) run back-to-back on the
     PE with chunk-interleaved w/x DMA underneath.
  2. Attention per (q-half, head, key-tile), scalar-engine-bound at one exp
     ([128x1024] PSUM->bf16) per slot.  attn@V runs ONE SLOT BehinD its exp so
     every PE instruction in a slot has pre-satisfied dependencies (no sem
     round trip on the critical path); the next head's first scores matmul is
     emitted two slots early for the same reason.  Leftover projection work
     (q-half-1 QT during half-0, half-0 output projection during half-1) is
     injected one micro-step per slot to keep the PE dense for HAM.
  3. Heads run hp=1 first so the last head's softmax tail (done in two
     512-column chunks to cut its latency) ends with a direct vector multiply,
     then the half-1 output projection drains.  Host sums the two half-head
     partials and adds the bias.
"""

import sys

if "/opt/trn_rl_repo" not in sys.path:
    sys.path.insert(0, "/opt/trn_rl_repo")

import numpy as np

B, S, D = 4, 2048, 1024
H, HD = 16, 64
P = 128
DK = D // P            # 8 contraction chunks for the projections
NKT = S // P           # 16 token tiles
QB = 512
DH = 512               # head dims per core (8 heads)
NDC = DH // P          # 4 dout chunks per core
NHC = 8                # heads per core
VW = HD + 1            # V columns per head incl. the ones column
HB = 1024              # q-half width
NCORES = 8

_PROG = [None]


def _build():
    import concourse.mybir as mybir
    import concourse.tile as tile
    from concourse import bacc

    f32 = mybir.dt.float32
    bf16 = mybir.dt.bfloat16
    Exp = mybir.ActivationFunctionType.Exp

    nc = bacc.Bacc("TRN2", target_bir_lowering=False, debug=False)
    xq = nc.dram_tensor("xq", [D, S], bf16, kind="ExternalInput").ap()
    xk = nc.dram_tensor("xk", [D, S], bf16, kind="ExternalInput").ap()
    xv = nc.dram_tensor("xv", [D, S], bf16, kind="ExternalInput").ap()
    wq = nc.dram_tensor("wq", [D, DH], bf16, kind="ExternalInput").ap()
    wk = nc.dram_tensor("wk", [D, DH], bf16, kind="ExternalInput").ap()
    wv = nc.dram_tensor("wv", [D, DH], bf16, kind="ExternalInput").ap()
    wo = nc.dram_tensor("wo", [DH, D], bf16, kind="ExternalInput").ap()
    part = nc.dram_tensor("part", [S, D], f32, kind="ExternalOutput").ap()

    xq_v = xq.rearrange("(c p) s -> p c s", p=P)
    xk_v = xk.rearrange("(c p) s -> p c s", p=P)
    xv_v = xv.rearrange("(c p) s -> p c s", p=P)

    with tile.TileContext(nc) as tc:
        with tc.tile_pool(name="big", bufs=1) as big, tc.tile_pool(name="wp", bufs=4) as wp:
            QT = big.tile([P, NDC, S], bf16, tag="QT")
            KT = big.tile([P, NDC, S], bf16, tag="KT")
            V = big.tile([P, NKT, NHC * VW], bf16, tag="V")
            outT = big.tile([P, NDC, S], bf16, tag="outT")

            # weight tiles live for the whole kernel (wq is reused by the
            # q-half-1 filler projection during attention)
            wk_t = wp.tile([P, DK, DH], bf16, tag="w", name="wk")
            wq_t = wp.tile([P, DK, DH], bf16, tag="w", name="wq")
            wv_t = wp.tile([P, DK, DH], bf16, tag="w", name="wv")
            wo_t = wp.tile([P, NDC, D], bf16, tag="w", name="wo")

            # scores PSUM pool lives at the outer level so the first two
            # scores matmuls can be emitted during the projection phase
            scp = tc.alloc_tile_pool(name="sc", bufs=2, space="PSUM")

            def emit_scores(kt, hc, r0, c0):
                sct = scp.tile([P, HB], f32, tag="sc")
                for j in range(2):
                    nc.tensor.matmul(
                        sct[:, j * QB : (j + 1) * QB],
                        KT[r0 : r0 + 64, hc, kt * P : (kt + 1) * P],
                        QT[
                            r0 : r0 + 64,
                            hc,
                            c0 + j * QB : c0 + (j + 1) * QB,
                        ],
                        start=True,
                        stop=True,
                    )
                return sct

            # hp=1 heads first: the final head's tail then has no DMA hop
            order = [1, 3, 5, 7, 0, 2, 4, 6]
            slots = [
                (half, h, kt)
                for half in (0, 1)
                for h in order
                for kt in range(NKT)
            ]

            def scores_for_slot(i):
                half, h, kt = slots[i]
                return emit_scores(kt, h // 2, 64 * (h % 2), half * HB)

            # ---- pre-attention projections -------------------------------
            # x is loaded as full-row [P, S] tiles (one per 128-d chunk):
            # 4 KB DMA lines instead of 1 KB quadruple the per-queue DMA
            # efficiency, and each row is reused by all four q-blocks.
            with (
                tc.tile_pool(name="xr", bufs=16) as xr,
                tc.tile_pool(name="pp", bufs=4, space="PSUM") as pp,
            ):
                def load_w(w_t, w_dram):
                    w_v = w_dram.rearrange("(c p) m -> p c m", p=P)
                    for dk in range(DK):
                        nc.sync.dma_start(w_t[:, dk], w_v[:, dk])

                def load_x_rows(x_view, nm, w_pair=None):
                    xts = []
                    for dk in range(DK):
                        if w_pair is not None:
                            w_t, w_v = w_pair
                            nc.sync.dma_start(w_t[:, dk], w_v[:, dk])
                        xt = xr.tile([P, S], bf16, tag="xr", name=f"xr_{nm}{dk}")
                        nc.sync.dma_start(xt[:], x_view[:, dk, :])
                        xts.append(xt)
                    return xts

                def proj_T(xts, w_t, out_t, qbs):
                    for qb in qbs:
                        pts = [pp.tile([P, QB], f32, tag="pp", name=f"pp{i}") for i in range(NDC)]
                        for dk in range(DK):
                            for dc in range(NDC):
                                nc.tensor.matmul(
                                    pts[dc][:],
                                    w_t[:, dk, dc * P : (dc + 1) * P],
                                    xts[dk][:, qb * QB : (qb + 1) * QB],
                                    start=(dk == 0),
                                    stop=(dk == DK - 1),
                                )
                        for dc in range(NDC):
                            dst = out_t[:, dc, qb * QB : (qb + 1) * QB]
                            if dc % 2 == 0:
                                nc.vector.tensor_copy(dst, pts[dc][:])
                            else:
                                nc.scalar.copy(dst, pts[dc][:])

                wk_v = wk.rearrange("(c p) m -> p c m", p=P)
                wq_v = wq.rearrange("(c p) m -> p c m", p=P)
                xk_rows = load_x_rows(xk_v, "k", w_pair=(wk_t, wk_v))
                proj_T(xk_rows, wk_t, KT, [0])
                xq_rows = load_x_rows(xq_v, "q", w_pair=(wq_t, wq_v))
                proj_T(xk_rows, wk_t, KT, [1, 2, 3])
                proj_T(xq_rows, wq_t, QT, [0])
                load_w(wv_t, wv)
                proj_T(xq_rows, wq_t, QT, [1])

                # the attention stream's first two scores run during the V
                # projection so the scalar engine starts exp before it ends
                sct_ring = {0: scores_for_slot(0), 1: scores_for_slot(1)}

                # V projection (tokens-on-partitions) + ones columns
                nc.vector.memset(V[:], 1.0)
                xv_rows = load_x_rows(xv_v, "v")
                nc.sync.dma_start(
                    wo_t[:], wo.rearrange("(c p) m -> p c m", p=P)
                )
                for qb in range(4):
                    pts = [pp.tile([P, DH], f32, tag="pp", name=f"ppv{i}") for i in range(QB // P)]
                    for dk in range(DK):
                        for kt_in in range(QB // P):
                            nc.tensor.matmul(
                                pts[kt_in][:],
                                xv_rows[dk][:, qb * QB + kt_in * P : qb * QB + (kt_in + 1) * P],
                                wv_t[:, dk, :],
                                start=(dk == 0),
                                stop=(dk == DK - 1),
                            )
                    for kt_in in range(QB // P):
                        kt = qb * (QB // P) + kt_in
                        nc.vector.tensor_copy(
                            V[:, kt].rearrange("p (h c) -> p h c", c=VW)[
                                :, :, 0:HD
                            ],
                            pts[kt_in][:].rearrange("p (h c) -> p h c", c=HD),
                        )

            # ---- attention: one uniform 256-slot stream -------------------
            # slot i: exp(i) on the scalar engine; PE emits scores(i+2),
            # one filler micro-step, and attnv(i-2).  Every PE instruction in
            # a slot has pre-satisfied dependencies, including across head
            # boundaries, so the stream runs at the exp cadence.
            with (
                tc.tile_pool(name="attn", bufs=6) as attnp,
                tc.tile_pool(name="tail", bufs=1) as tailp,
                tc.tile_pool(name="asbp", bufs=2) as asbp,
                tc.tile_pool(name="stage", bufs=6) as stage,
                tc.tile_pool(name="xf", bufs=8) as xf,
                tc.tile_pool(name="acc", bufs=1, space="PSUM") as accp,
                tc.tile_pool(name="fillpp", bufs=2, space="PSUM") as fillp,
            ):
                # -- filler micro-step generators --
                def qt23_steps():
                    """Project QT for q-half 1 (qb 2,3), in ~0.5us steps."""
                    xts_all = {}
                    cur = {}

                    def dma_qb(dk0):
                        def go():
                            for dk in (dk0, dk0 + 1, dk0 + 2, dk0 + 3):
                                xt = xf.tile([P, HB], bf16, tag="xf", name=f"xf{dk}")
                                nc.sync.dma_start(
                                    xt[:], xq_v[:, dk, HB : 2 * HB]
                                )
                                xts_all[dk] = xt
                        return go

                    def mm_step(qb, dc, dk):
                        def go():
                            if dk == 0:
                                cur[(qb, dc)] = fillp.tile([P, QB], f32, tag="fp", name=f"fq{qb}_{dc}")
                            nc.tensor.matmul(
                                cur[(qb, dc)][:],
                                wq_t[:, dk, dc * P : (dc + 1) * P],
                                xts_all[dk][:, (qb - 2) * QB : (qb - 1) * QB],
                                start=(dk == 0),
                                stop=(dk == DK - 1),
                            )
                        return go

                    def copy_step(qb, dc):
                        def go():
                            nc.vector.tensor_copy(
                                QT[:, dc, qb * QB : (qb + 1) * QB],
                                cur[(qb, dc)][:],
                            )
                        return go

                    yield dma_qb(0)
                    yield dma_qb(4)
                    for qb in (2, 3):
                        for dc in range(NDC):
                            for dk in range(DK):
                                yield mm_step(qb, dc, dk)
                            yield copy_step(qb, dc)

                def oproj_steps(qts):
                    """Output projection for token tiles qts, in ~0.5us steps."""
                    cur = {}

                    def mm_step(qt, do, dc):
                        def go():
                            if dc == 0:
                                cur[(qt, do)] = fillp.tile([P, QB], f32, tag="fp", name=f"fo{qt}_{do}")
                            nc.tensor.matmul(
                                cur[(qt, do)][:],
                                outT[:, dc, qt * P : (qt + 1) * P],
                                wo_t[:, dc, do * QB : (do + 1) * QB],
                                start=(dc == 0),
                                stop=(dc == NDC - 1),
                            )
                        return go

                    def out_step(qt, do):
                        def go():
                            st = stage.tile([P, QB], f32, tag="st", name=f"st{qt}_{do}")
                            nc.vector.tensor_copy(st[:], cur[(qt, do)][:])
                            nc.sync.dma_start(
                                part[
                                    qt * P : (qt + 1) * P,
                                    do * QB : (do + 1) * QB,
                                ],
                                st[:],
                            )
                        return go

                    for qt in qts:
                        for do in range(2):
                            for dc in range(NDC):
                                yield mm_step(qt, do, dc)
                            yield out_step(qt, do)

                def emit_attnv(acc, h, kt, at_t):
                    for j in range(2):
                        nc.tensor.matmul(
                            acc[0:VW, j * QB : (j + 1) * QB],
                            V[:, kt, h * VW : (h + 1) * VW],
                            at_t[:, j * QB : (j + 1) * QB],
                            start=(kt == 0),
                            stop=(kt == NKT - 1),
                        )

                def emit_tail(acc, hp, hc, c0):
                    # one full-width copy frees the PSUM accumulator fast for
                    # the next head; the rest runs in two 512-column chunks so
                    # the last head's outT is available with low latency
                    asb = asbp.tile([96, HB], f32, tag="asb")
                    nc.vector.tensor_copy(asb[0:VW, :], acc[0:VW, :])
                    for ch in range(2):
                        cl = slice(ch * QB, (ch + 1) * QB)
                        bc = tailp.tile([64, QB], f32, tag="bc", name=f"bc{ch}")
                        nc.vector.stream_shuffle(
                            bc[0:32, :], asb[64:96, cl], [0] * 32
                        )
                        nc.vector.stream_shuffle(
                            bc[32:64, :], asb[64:96, cl], [0] * 32
                        )
                        rec = tailp.tile([64, QB], f32, tag="rec", name=f"rec{ch}")
                        scr = tailp.tile([64, QB], f32, tag="scr", name=f"scr{ch}")
                        nc.vector.reciprocal_approx_accurate(
                            rec[:], bc[:], scr[:]
                        )
                        dst = outT[
                            64 * hp : 64 * hp + 64,
                            hc,
                            c0 + ch * QB : c0 + (ch + 1) * QB,
                        ]
                        if hp == 0:
                            nc.vector.tensor_mul(dst, asb[0:HD, cl], rec[:])
                        else:
                            tmp = tailp.tile([64, QB], bf16, tag="tmp", name=f"tmp{ch}")
                            nc.vector.tensor_mul(tmp[:], asb[0:HD, cl], rec[:])
                            nc.sync.dma_start(dst, tmp[:])

                iters = {0: qt23_steps(), 1: oproj_steps(range(8))}
                NSLOT = len(slots)
                at_ring = {}
                acc_cur = [None]

                def do_attnv(j):
                    half2, h2, kt2 = slots[j]
                    if kt2 == 0:
                        acc_cur[0] = accp.tile([P, HB], f32, tag="acc", name=f"acc{j // NKT}")
                    emit_attnv(acc_cur[0], h2, kt2, at_ring.pop(j))
                    if kt2 == NKT - 1:
                        emit_tail(acc_cur[0], h2 % 2, h2 // 2, half2 * HB)

                for i in range(NSLOT):
                    half, h, kt = slots[i]
                    at_t = attnp.tile([P, HB], bf16, tag="attn", name=f"at{i % 6}")
                    nc.scalar.activation(at_t[:], sct_ring.pop(i)[:], Exp)
                    at_ring[i] = at_t
                    # PE order inside the slot: attnv and filler first (their
                    # deps are pre-satisfied), scores last -- scores(i+2)
                    # waits for exp(i) to free its PSUM slot and would
                    # head-of-line block the PE queue otherwise
                    if i >= 2:
                        do_attnv(i - 2)
                    if 2 <= (i % NKT) <= 13:
                        s = next(iters[half], None)
                        if s is not None:
                            s()
                    if i + 2 < NSLOT:
                        sct_ring[i + 2] = scores_for_slot(i + 2)
                for j in (NSLOT - 2, NSLOT - 1):
                    do_attnv(j)

                # leftovers: half-0 output projection stragglers, then the
                # half-1 output projection
                for s in iters[1]:
                    s()
                for s in oproj_steps(range(8, 16)):
                    s()

            scp.release()

    nc.compile()
    return nc


def _get_prog():
    if _PROG[0] is None:
        _PROG[0] = _build()
    return _PROG[0]


def make_in_maps(query, key, value, Wq, Wk, Wv, Wo):
    import ml_dtypes

    bf16 = ml_dtypes.bfloat16
    scale = np.float32(1.0 / np.sqrt(D))
    Wq_s = (np.asarray(Wq, np.float32) * scale).astype(bf16)
    Wk_s = np.asarray(Wk, np.float32).astype(bf16)
    Wv_s = np.asarray(Wv, np.float32).astype(bf16)
    Wo_s = np.asarray(Wo, np.float32).astype(bf16)
    in_maps = []
    for b in range(B):
        xqT = np.ascontiguousarray(np.asarray(query[b], np.float32).T.astype(bf16))
        xkT = np.ascontiguousarray(np.asarray(key[b], np.float32).T.astype(bf16))
        xvT = np.ascontiguousarray(np.asarray(value[b], np.float32).T.astype(bf16))
        for hh in range(2):
            sl = slice(hh * DH, (hh + 1) * DH)
            in_maps.append(
                {
                    "xq": xqT,
                    "xk": xkT,
                    "xv": xvT,
                    "wq": np.ascontiguousarray(Wq_s[:, sl]),
                    "wk": np.ascontiguousarray(Wk_s[:, sl]),
                    "wv": np.ascontiguousarray(Wv_s[:, sl]),
                    "wo": np.ascontiguousarray(Wo_s[sl, :]),
                }
            )
    return in_maps


def run(in_maps, trace=False, **kw):
    from concourse.bass_utils import run_bass_kernel_spmd

    nc = _get_prog()
    return run_bass_kernel_spmd(
        nc, in_maps, core_ids=list(range(NCORES)), trace=trace, **kw
    )


def kernel(query, key, value, Wq, Wk, Wv, Wo, bo):
    in_maps = make_in_maps(query, key, value, Wq, Wk, Wv, Wo)
    res = run(in_maps)
    bo = np.asarray(bo, np.float32)
    out = np.empty((B, S, D), np.float32)
    for b in range(B):
        out[b] = res.results[2 * b]["part"] + res.results[2 * b + 1]["part"] + bo
    return out


# revision 16
# speedup vs baseline: 1.0034x; 1.0034x over previous
"""Multi-head attention (B=4, S=2048, D=1024, H=16) on 8 trn2 NeuronCores.

Sharding: batch (4-way) x head-half (2-way).  Core c = 2*b + hh handles batch b
and heads hh*8 .. hh*8+7.  v3: all matmul operands bf16 (halves DMA, enables
fast weight load), scheduled so neither the PE HAM clock-gate nor the
exp->attnv->scores semaphore loop limits the attention cadence:

  1. KT proj (full), QT proj (q-half 0), V proj (full) run back-to-back on the
     PE with chunk-interleaved w/x DMA underneath.
  2. Attention per (q-half, head, key-tile), scalar-engine-bound at one exp
     ([128x1024] PSUM->bf16) per slot.  attn@V runs ONE SLOT BehinD its exp so
     every PE instruction in a slot has pre-satisfied dependencies (no sem
     round trip on the critical path); the next head's first scores matmul is
     emitted two slots early for the same reason.  Leftover projection work
     (q-half-1 QT during half-0, half-0 output projection during half-1) is
     injected one micro-step per slot to keep the PE dense for HAM.
  3. Heads run hp=1 first so the last head's softmax tail (done in two
     512-column chunks to cut its latency) ends with a direct vector multiply,
     then the half-1 output projection drains.  Host sums the two half-head
     partials and adds the bias.
"""

import sys

if "/opt/trn_rl_repo" not in sys.path:
    sys.path.insert(0, "/opt/trn_rl_repo")

import numpy as np

B, S, D = 4, 2048, 1024
H, HD = 16, 64
P = 128
DK = D // P            # 8 contraction chunks for the projections
NKT = S // P           # 16 token tiles
QB = 512
DH = 512               # head dims per core (8 heads)
NDC = DH // P          # 4 dout chunks per core
NHC = 8                # heads per core
VW = HD + 1            # V columns per head incl. the ones column
HB = 1024              # q-half width
NCORES = 8

_PROG = [None]


def _build():
    import concourse.mybir as mybir
    import concourse.tile as tile
    from concourse import bacc

    f32 = mybir.dt.float32
    bf16 = mybir.dt.bfloat16
    Exp = mybir.ActivationFunctionType.Exp

    nc = bacc.Bacc("TRN2", target_bir_lowering=False, debug=False)
    xq = nc.dram_tensor("xq", [D, S], bf16, kind="ExternalInput").ap()
    xk = nc.dram_tensor("xk", [D, S], bf16, kind="ExternalInput").ap()
    xv = nc.dram_tensor("xv", [D, S], bf16, kind="ExternalInput").ap()
    wq = nc.dram_tensor("wq", [D, DH], bf16, kind="ExternalInput").ap()
    wk = nc.dram_tensor("wk", [D, DH], bf16, kind="ExternalInput").ap()
    wv = nc.dram_tensor("wv", [D, DH], bf16, kind="ExternalInput").ap()
    wo = nc.dram_tensor("wo", [DH, D], bf16, kind="ExternalInput").ap()
    part = nc.dram_tensor("part", [S, D], f32, kind="ExternalOutput").ap()

    xq_v = xq.rearrange("(c p) s -> p c s", p=P)
    xk_v = xk.rearrange("(c p) s -> p c s", p=P)
    xv_v = xv.rearrange("(c p) s -> p c s", p=P)

    with tile.TileContext(nc) as tc:
        with tc.tile_pool(name="big", bufs=1) as big, tc.tile_pool(name="wp", bufs=4) as wp:
            QT = big.tile([P, NDC, S], bf16, tag="QT")
            KT = big.tile([P, NDC, S], bf16, tag="KT")
            V = big.tile([P, NKT, NHC * VW], bf16, tag="V")
            outT = big.tile([P, NDC, S], bf16, tag="outT")

            # weight tiles live for the whole kernel (wq is reused by the
            # q-half-1 filler projection during attention)
            wk_t = wp.tile([P, DK, DH], bf16, tag="w", name="wk")
            wq_t = wp.tile([P, DK, DH], bf16, tag="w", name="wq")
            wv_t = wp.tile([P, DK, DH], bf16, tag="w", name="wv")
            wo_t = wp.tile([P, NDC, D], bf16, tag="w", name="wo")

            # scores PSUM pool lives at the outer level so the first two
            # scores matmuls can be emitted during the projection phase
            scp = tc.alloc_tile_pool(name="sc", bufs=2, space="PSUM")

            def emit_scores(kt, hc, r0, c0):
                sct = scp.tile([P, HB], f32, tag="sc")
                for j in range(2):
                    nc.tensor.matmul(
                        sct[:, j * QB : (j + 1) * QB],
                        KT[r0 : r0 + 64, hc, kt * P : (kt + 1) * P],
                        QT[
                            r0 : r0 + 64,
                            hc,
                            c0 + j * QB : c0 + (j + 1) * QB,
                        ],
                        start=True,
                        stop=True,
                    )
                return sct

            # hp=1 heads first: the final head's tail then has no DMA hop
            order = [1, 3, 5, 7, 0, 2, 4, 6]
            slots = [
                (half, h, kt)
                for half in (0, 1)
                for h in order
                for kt in range(NKT)
            ]

            def scores_for_slot(i):
                half, h, kt = slots[i]
                return emit_scores(kt, h // 2, 64 * (h % 2), half * HB)

            # ---- pre-attention projections -------------------------------
            # x is loaded as full-row [P, S] tiles (one per 128-d chunk):
            # 4 KB DMA lines instead of 1 KB quadruple the per-queue DMA
            # efficiency, and each row is reused by all four q-blocks.
            with (
                tc.tile_pool(name="xr", bufs=16) as xr,
                tc.tile_pool(name="pp", bufs=4, space="PSUM") as pp,
            ):
                def load_w(w_t, w_dram):
                    w_v = w_dram.rearrange("(c p) m -> p c m", p=P)
                    for dk in range(DK):
                        nc.sync.dma_start(w_t[:, dk], w_v[:, dk])

                def load_x_rows(x_view, nm, w_pair=None):
                    xts = []
                    for dk in range(DK):
                        if w_pair is not None:
                            w_t, w_v = w_pair
                            nc.sync.dma_start(w_t[:, dk], w_v[:, dk])
                        xt = xr.tile([P, S], bf16, tag="xr", name=f"xr_{nm}{dk}")
                        nc.sync.dma_start(xt[:], x_view[:, dk, :])
                        xts.append(xt)
                    return xts

                def proj_T(xts, w_t, out_t, qbs):
                    for qb in qbs:
                        pts = [pp.tile([P, QB], f32, tag="pp", name=f"pp{i}") for i in range(NDC)]
                        for dk in range(DK):
                            for dc in range(NDC):
                                nc.tensor.matmul(
                                    pts[dc][:],
                                    w_t[:, dk, dc * P : (dc + 1) * P],
                                    xts[dk][:, qb * QB : (qb + 1) * QB],
                                    start=(dk == 0),
                                    stop=(dk == DK - 1),
                                )
                        for dc in range(NDC):
                            dst = out_t[:, dc, qb * QB : (qb + 1) * QB]
                            if dc % 2 == 0:
                                nc.vector.tensor_copy(dst, pts[dc][:])
                            else:
                                nc.scalar.copy(dst, pts[dc][:])

                wk_v = wk.rearrange("(c p) m -> p c m", p=P)
                wq_v = wq.rearrange("(c p) m -> p c m", p=P)
                xk_rows = load_x_rows(xk_v, "k", w_pair=(wk_t, wk_v))
                proj_T(xk_rows, wk_t, KT, [0])
                xq_rows = load_x_rows(xq_v, "q", w_pair=(wq_t, wq_v))
                proj_T(xk_rows, wk_t, KT, [1, 2, 3])
                proj_T(xq_rows, wq_t, QT, [0])
                load_w(wv_t, wv)
                proj_T(xq_rows, wq_t, QT, [1])

                # the attention stream's first two scores run during the V
                # projection so the scalar engine starts exp before it ends
                sct_ring = {0: scores_for_slot(0), 1: scores_for_slot(1)}

                # V projection (tokens-on-partitions) + ones columns
                nc.vector.memset(V[:], 1.0)
                xv_rows = load_x_rows(xv_v, "v")
                nc.sync.dma_start(
                    wo_t[:], wo.rearrange("(c p) m -> p c m", p=P)
                )
                for qb in range(4):
                    pts = [pp.tile([P, DH], f32, tag="pp", name=f"ppv{i}") for i in range(QB // P)]
                    for dk in range(DK):
                        for kt_in in range(QB // P):
                            nc.tensor.matmul(
                                pts[kt_in][:],
                                xv_rows[dk][:, qb * QB + kt_in * P : qb * QB + (kt_in + 1) * P],
                                wv_t[:, dk, :],
                                start=(dk == 0),
                                stop=(dk == DK - 1),
                            )
                    for kt_in in range(QB // P):
                        kt = qb * (QB // P) + kt_in
                        nc.vector.tensor_copy(
                            V[:, kt].rearrange("p (h c) -> p h c", c=VW)[
                                :, :, 0:HD
                            ],
                            pts[kt_in][:].rearrange("p (h c) -> p h c", c=HD),
                        )

            # ---- attention: one uniform 256-slot stream -------------------
            # slot i: exp(i) on the scalar engine; PE emits scores(i+2),
            # one filler micro-step, and attnv(i-2).  Every PE instruction in
            # a slot has pre-satisfied dependencies, including across head
            # boundaries, so the stream runs at the exp cadence.
            with (
                tc.tile_pool(name="attn", bufs=20) as attnp,
                tc.tile_pool(name="tail", bufs=1) as tailp,
                tc.tile_pool(name="asbp", bufs=2) as asbp,
                tc.tile_pool(name="stage", bufs=6) as stage,
                tc.tile_pool(name="xf", bufs=8) as xf,
                tc.tile_pool(name="acc", bufs=1, space="PSUM") as accp,
                tc.tile_pool(name="fillpp", bufs=2, space="PSUM") as fillp,
            ):
                # -- filler micro-step generators --
                def qt23_steps():
                    """Project QT for q-half 1 (qb 2,3), in ~0.5us steps."""
                    xts_all = {}
                    cur = {}

                    def dma_qb(dk0):
                        def go():
                            for dk in (dk0, dk0 + 1, dk0 + 2, dk0 + 3):
                                xt = xf.tile([P, HB], bf16, tag="xf", name=f"xf{dk}")
                                nc.sync.dma_start(
                                    xt[:], xq_v[:, dk, HB : 2 * HB]
                                )
                                xts_all[dk] = xt
                        return go

                    def mm_step(qb, dc, dk):
                        def go():
                            if dk == 0:
                                cur[(qb, dc)] = fillp.tile([P, QB], f32, tag="fp", name=f"fq{qb}_{dc}")
                            nc.tensor.matmul(
                                cur[(qb, dc)][:],
                                wq_t[:, dk, dc * P : (dc + 1) * P],
                                xts_all[dk][:, (qb - 2) * QB : (qb - 1) * QB],
                                start=(dk == 0),
                                stop=(dk == DK - 1),
                            )
                        return go

                    def copy_step(qb, dc):
                        def go():
                            nc.vector.tensor_copy(
                                QT[:, dc, qb * QB : (qb + 1) * QB],
                                cur[(qb, dc)][:],
                            )
                        return go

                    yield dma_qb(0)
                    yield dma_qb(4)
                    for qb in (2, 3):
                        for dc in range(NDC):
                            for dk in range(DK):
                                yield mm_step(qb, dc, dk)
                            yield copy_step(qb, dc)

                def oproj_steps(qts):
                    """Output projection for token tiles qts, in ~0.5us steps."""
                    cur = {}

                    def mm_step(qt, do, dc):
                        def go():
                            if dc == 0:
                                cur[(qt, do)] = fillp.tile([P, QB], f32, tag="fp", name=f"fo{qt}_{do}")
                            nc.tensor.matmul(
                                cur[(qt, do)][:],
                                outT[:, dc, qt * P : (qt + 1) * P],
                                wo_t[:, dc, do * QB : (do + 1) * QB],
                                start=(dc == 0),
                                stop=(dc == NDC - 1),
                            )
                        return go

                    def out_step(qt, do):
                        def go():
                            st = stage.tile([P, QB], f32, tag="st", name=f"st{qt}_{do}")
                            nc.vector.tensor_copy(st[:], cur[(qt, do)][:])
                            nc.sync.dma_start(
                                part[
                                    qt * P : (qt + 1) * P,
                                    do * QB : (do + 1) * QB,
                                ],
                                st[:],
                            )
                        return go

                    for qt in qts:
                        for do in range(2):
                            for dc in range(NDC):
                                yield mm_step(qt, do, dc)
                            yield out_step(qt, do)

                def emit_attnv(acc, h, kt, at_t):
                    for j in range(2):
                        nc.tensor.matmul(
                            acc[0:VW, j * QB : (j + 1) * QB],
                            V[:, kt, h * VW : (h + 1) * VW],
                            at_t[:, j * QB : (j + 1) * QB],
                            start=(kt == 0),
                            stop=(kt == NKT - 1),
                        )

                def emit_tail(acc, hp, hc, c0):
                    # one full-width copy frees the PSUM accumulator fast for
                    # the next head; the rest runs in two 512-column chunks so
                    # the last head's outT is available with low latency
                    asb = asbp.tile([96, HB], f32, tag="asb")
                    nc.vector.tensor_copy(asb[0:VW, :], acc[0:VW, :])
                    for ch in range(2):
                        cl = slice(ch * QB, (ch + 1) * QB)
                        bc = tailp.tile([64, QB], f32, tag="bc", name=f"bc{ch}")
                        nc.vector.stream_shuffle(
                            bc[0:32, :], asb[64:96, cl], [0] * 32
                        )
                        nc.vector.stream_shuffle(
                            bc[32:64, :], asb[64:96, cl], [0] * 32
                        )
                        rec = tailp.tile([64, QB], f32, tag="rec", name=f"rec{ch}")
                        scr = tailp.tile([64, QB], f32, tag="scr", name=f"scr{ch}")
                        nc.vector.reciprocal_approx_accurate(
                            rec[:], bc[:], scr[:]
                        )
                        dst = outT[
                            64 * hp : 64 * hp + 64,
                            hc,
                            c0 + ch * QB : c0 + (ch + 1) * QB,
                        ]
                        if hp == 0:
                            nc.vector.tensor_mul(dst, asb[0:HD, cl], rec[:])
                        else:
                            tmp = tailp.tile([64, QB], bf16, tag="tmp", name=f"tmp{ch}")
                            nc.vector.tensor_mul(tmp[:], asb[0:HD, cl], rec[:])
                            nc.sync.dma_start(dst, tmp[:])

                iters = {0: qt23_steps(), 1: oproj_steps(range(8))}
                NSLOT = len(slots)
                at_ring = {}
                acc_cur = [None]

                def do_attnv(j):
                    half2, h2, kt2 = slots[j]
                    if kt2 == 0:
                        acc_cur[0] = accp.tile([P, HB], f32, tag="acc", name=f"acc{j // NKT}")
                    emit_attnv(acc_cur[0], h2, kt2, at_ring.pop(j))
                    if kt2 == NKT - 1:
                        emit_tail(acc_cur[0], h2 % 2, h2 // 2, half2 * HB)

                for i in range(NSLOT):
                    half, h, kt = slots[i]
                    at_t = attnp.tile([P, HB], bf16, tag="attn", name=f"at{i % 20}")
                    nc.scalar.activation(at_t[:], sct_ring.pop(i)[:], Exp)
                    at_ring[i] = at_t
                    # PE order inside the slot: attnv and filler first (their
                    # deps are pre-satisfied), scores last -- scores(i+2)
                    # waits for exp(i) to free its PSUM slot and would
                    # head-of-line block the PE queue otherwise
                    if i >= 2:
                        do_attnv(i - 2)
                    if 2 <= (i % NKT) <= 13:
                        s = next(iters[half], None)
                        if s is not None:
                            s()
                    if i + 2 < NSLOT:
                        sct_ring[i + 2] = scores_for_slot(i + 2)
                for j in (NSLOT - 2, NSLOT - 1):
                    do_attnv(j)

                # leftovers: half-0 output projection stragglers, then the
                # half-1 output projection
                for s in iters[1]:
                    s()
                for s in oproj_steps(range(8, 16)):
                    s()

            scp.release()

    nc.compile()
    return nc


def _get_prog():
    if _PROG[0] is None:
        _PROG[0] = _build()
    return _PROG[0]


def make_in_maps(query, key, value, Wq, Wk, Wv, Wo):
    import ml_dtypes

    bf16 = ml_dtypes.bfloat16
    scale = np.float32(1.0 / np.sqrt(D))
    Wq_s = (np.asarray(Wq, np.float32) * scale).astype(bf16)
    Wk_s = np.asarray(Wk, np.float32).astype(bf16)
    Wv_s = np.asarray(Wv, np.float32).astype(bf16)
    Wo_s = np.asarray(Wo, np.float32).astype(bf16)
    in_maps = []
    for b in range(B):
        xqT = np.ascontiguousarray(np.asarray(query[b], np.float32).T.astype(bf16))
        xkT = np.ascontiguousarray(np.asarray(key[b], np.float32).T.astype(bf16))
        xvT = np.ascontiguousarray(np.asarray(value[b], np.float32).T.astype(bf16))
        for hh in range(2):
            sl = slice(hh * DH, (hh + 1) * DH)
            in_maps.append(
                {
                    "xq": xqT,
                    "xk": xkT,
                    "xv": xvT,
                    "wq": np.ascontiguousarray(Wq_s[:, sl]),
                    "wk": np.ascontiguousarray(Wk_s[:, sl]),
                    "wv": np.ascontiguousarray(Wv_s[:, sl]),
                    "wo": np.ascontiguousarray(Wo_s[sl, :]),
                }
            )
    return in_maps


def run(in_maps, trace=False, **kw):
    from concourse.bass_utils import run_bass_kernel_spmd

    nc = _get_prog()
    return run_bass_kernel_spmd(
        nc, in_maps, core_ids=list(range(NCORES)), trace=trace, **kw
    )


def kernel(query, key, value, Wq, Wk, Wv, Wo, bo):
    in_maps = make_in_maps(query, key, value, Wq, Wk, Wv, Wo)
    res = run(in_maps)
    bo = np.asarray(bo, np.float32)
    out = np.empty((B, S, D), np.float32)
    for b in range(B):
        out[b] = res.results[2 * b]["part"] + res.results[2 * b + 1]["part"] + bo
    return out
